# revision 47
# baseline (speedup 1.0000x reference)
"""MoE (DBRX-style FFN) kernel for 8 Trainium2 NeuronCores.

Strategy (expert parallelism per the sharding hint):
  - 16 experts sharded 2-per-core; the small router is replicated.
  - Host-side dispatch: the host runs the router only to decide which tokens
    go to which expert (token dispatch), gathers each expert's tokens into a
    transposed [H, C] block, and lays out the expert weights in DMA-friendly
    tiles.  All floating-point math that produces the outputs runs on device.
  - Device per core: router softmax for its 256-token slice of the `weights`
    output; per-expert combine weights (softmax + top-4 + L1 normalize)
    recomputed on device from the gathered tokens; h = x@w1, v = x@v1 in
    fp32r; g = silu(h)*v (bf16); y = g@w2 (bf16); y rows scaled by combine
    weight and scattered (indirect DMA) into a [T, H] partial buffer;
    ReduceScatter over the 8 cores yields each core's 256-token output slice.
  - Host concatenates the 8 slices (pure unshard).
"""

import os
import sys

sys.path.insert(0, "/opt/trn_rl_repo")

import numpy as np

import concourse.bacc as bacc
import concourse.bass as bass
import concourse.mybir as mybir
import concourse.tile as tile
from concourse.bass import IndirectOffsetOnAxis
from concourse.bass_utils import run_bass_kernel_spmd

F32 = mybir.dt.float32
F32R = mybir.dt.float32r
BF16 = mybir.dt.bfloat16
I32 = mybir.dt.int32

# Problem dims (hardcoded per spec)
T = 2048
H = 2048
F = 4096
E = 16
TOPK = 4

NCORES = 8
EPC = E // NCORES  # experts per core
C = 640            # token capacity per expert (mean load is 512, std ~20)
TS = T // NCORES   # output tokens per core

LAST_RESULT = None  # test harness reads exec_time_ns from here
USE_NATIVE_SILU = True  # CoreSim lacks Silu; sim tests flip this to False


# ----------------------------------------------------------------------------
# Device program
# ----------------------------------------------------------------------------

def build_nc(trace_label=""):
    HT = H // 128
    FT = F // 128
    CT = C // 128
    TT = TS // 128
    # phase-1 moving-dim chunks (keep >=256 so fp32r runs at full rate)
    half = C // 2
    P1_CHUNKS = [(0, half), (half, C - half)]
    HC = 512 if H % 512 == 0 else H
    HCN = H // HC

    nc = bacc.Bacc(
        "TRN2", target_bir_lowering=False, debug=False, num_devices=NCORES
    )

    xg_d = nc.dram_tensor("xg", [EPC, H, C], F32, kind="ExternalInput")
    w1h_d = nc.dram_tensor("w1h", [EPC, FT, 128, H], BF16, kind="ExternalInput")
    v1h_d = nc.dram_tensor("v1h", [EPC, FT, 128, H], BF16, kind="ExternalInput")
    w2_d = nc.dram_tensor("w2", [EPC, F, H], BF16, kind="ExternalInput")
    sidx_d = nc.dram_tensor("sidx", [EPC, C], I32, kind="ExternalInput")
    esel_d = nc.dram_tensor("esel", [EPC, 128, E], F32, kind="ExternalInput")
    kmask_d = nc.dram_tensor("kmask", [EPC, C, E], F32, kind="ExternalInput")
    xts_d = nc.dram_tensor("xts", [H, TS], F32, kind="ExternalInput")
    rwt_d = nc.dram_tensor("rwt", [H, E], F32, kind="ExternalInput")

    out_d = nc.dram_tensor("out_slice", [TS, H], F32, kind="ExternalOutput")
    wts_d = nc.dram_tensor("wts_slice", [TS, E], F32, kind="ExternalOutput")

    # two column halves so the first ReduceScatter overlaps phase-2 compute
    NHALF = 2 if (H // (512 if H % 512 == 0 else H)) % 2 == 0 else 1
    HHALF = H // NHALF
    acc_d = [
        nc.dram_tensor(f"acc{i}", [T + 128, HHALF], F32) for i in range(NHALF)
    ]  # +pad rows as scatter trash
    rs_d = [nc.dram_tensor(f"rs_out{i}", [TS, HHALF], F32) for i in range(NHALF)]

    with tile.TileContext(nc) as tc:
        with (
            tc.tile_pool(name="const", bufs=1) as constp,
            tc.tile_pool(name="psum_r", bufs=1, space="PSUM") as psum_r,
            tc.tile_pool(name="smax", bufs=2) as smaxp,
        ):
            # ---- constants ----
            rwt_sb = constp.tile([128, HT * E], F32, tag="rwt")
            nc.sync.dma_start(
                out=rwt_sb[:].rearrange("p (ht e) -> p ht e", e=E),
                in_=rwt_d.rearrange("(ht p) e -> p ht e", p=128),
            )
            esel_sb = constp.tile([128, EPC * E], F32, tag="esel")
            nc.sync.dma_start(
                out=esel_sb[:].rearrange("p (e q) -> p e q", q=E),
                in_=esel_d.rearrange("e p q -> p e q"),
            )
            sidx_sb = constp.tile([128, EPC * CT], I32, tag="sidx")
            nc.sync.dma_start(
                out=sidx_sb[:].rearrange("p (e ct) -> p e ct", ct=CT),
                in_=sidx_d.rearrange("e (ct p) -> p e ct", p=128),
            )
            cw_sb = constp.tile([128, EPC * CT], F32, tag="cw")

            # ---- zero the accumulator buffers ----
            zero_sb = constp.tile([128, H], F32, tag="zero")
            nc.vector.memset(zero_sb[:], 0.0)
            for r in range((T + 128) // 128):
                for i in range(NHALF):
                    nc.sync.dma_start(
                        out=acc_d[i][r * 128:(r + 1) * 128, :],
                        in_=zero_sb[:, :HHALF],
                    )

            def softmax16(ps, out_w):
                """ps: [128, E] psum logits -> out_w [128, E] sbuf softmax."""
                mx = smaxp.tile([128, 1], F32, tag="mx")
                nc.vector.tensor_reduce(
                    mx[:], ps[:], axis=mybir.AxisListType.X, op=mybir.AluOpType.max
                )
                negm = smaxp.tile([128, 1], F32, tag="negm")
                nc.vector.tensor_scalar_mul(negm[:], mx[:], -1.0)
                ex = out_w
                nc.scalar.activation(
                    ex[:], ps[:], mybir.ActivationFunctionType.Exp,
                    bias=negm[:, :1], scale=1.0,
                )
                sm = smaxp.tile([128, 1], F32, tag="sm")
                nc.vector.tensor_reduce(
                    sm[:], ex[:], axis=mybir.AxisListType.X, op=mybir.AluOpType.add
                )
                rc = smaxp.tile([128, 1], F32, tag="rc")
                nc.vector.reciprocal(rc[:], sm[:])
                nc.vector.tensor_scalar_mul(ex[:], ex[:], rc[:, :1])

            # ---- main expert pipeline ----
            with (
                tc.tile_pool(name="xgp", bufs=1) as xgp,
                tc.tile_pool(name="gp", bufs=1) as gp,
                tc.tile_pool(name="yp", bufs=1) as yp,
                tc.tile_pool(name="w1s", bufs=3) as w1s,
                tc.tile_pool(name="w2s", bufs=3) as w2s,
                tc.tile_pool(name="psAY", bufs=1, space="PSUM") as psAY,
                tc.tile_pool(name="silu", bufs=2) as silup,
            ):
                for e in range(EPC):
                    # load gathered tokens [H, C] -> [128, HT*C]
                    xg_sb = xgp.tile([128, HT * C], F32, tag="xg")
                    nc.sync.dma_start(
                        out=xg_sb[:].rearrange("p (ht c) -> p ht c", c=C),
                        in_=xg_d[e].rearrange("(ht p) c -> p ht c", p=128),
                    )
                    # bf16 copy feeds the big phase-1 matmuls (FWL-fast loads)
                    xgb_sb = xgp.tile([128, HT * C], BF16, tag="xgb")
                    nc.vector.tensor_copy(xgb_sb[:], xg_sb[:])

                    # combine weights for this expert's slots (on device)
                    for ct in range(CT):
                        ps = psum_r.tile([128, E], F32, tag="rps")
                        for ht in range(HT):
                            nc.tensor.matmul(
                                ps[:],
                                lhsT=xg_sb[:, ht * C + ct * 128: ht * C + (ct + 1) * 128],
                                rhs=rwt_sb[:, ht * E:(ht + 1) * E],
                                start=(ht == 0),
                                stop=(ht == HT - 1),
                            )
                        wfull = smaxp.tile([128, E], F32, tag="wfull")
                        softmax16(ps, wfull)
                        # top-4 selection comes from the host's dispatch mask,
                        # so selection is consistent with the scatter indices
                        # even for near-tied weights.
                        km = smaxp.tile([128, E], F32, tag="km")
                        nc.sync.dma_start(
                            out=km[:], in_=kmask_d[e, ct * 128:(ct + 1) * 128, :]
                        )
                        kept = smaxp.tile([128, E], F32, tag="kept")
                        nc.vector.tensor_tensor(
                            kept[:], wfull[:], km[:], op=mybir.AluOpType.mult
                        )
                        l1 = smaxp.tile([128, 1], F32, tag="l1")
                        nc.vector.tensor_reduce(
                            l1[:], kept[:], axis=mybir.AxisListType.X,
                            op=mybir.AluOpType.add,
                        )
                        num = smaxp.tile([128, E], F32, tag="num")
                        nc.vector.tensor_tensor(
                            num[:], kept[:], esel_sb[:, e * E:(e + 1) * E],
                            op=mybir.AluOpType.mult,
                        )
                        cwv = smaxp.tile([128, 1], F32, tag="cwv")
                        nc.vector.tensor_reduce(
                            cwv[:], num[:], axis=mybir.AxisListType.X,
                            op=mybir.AluOpType.add,
                        )
                        rc = smaxp.tile([128, 1], F32, tag="rc2")
                        nc.vector.reciprocal(rc[:], l1[:])
                        nc.vector.tensor_tensor(
                            cw_sb[:, e * CT + ct: e * CT + ct + 1], cwv[:], rc[:],
                            op=mybir.AluOpType.mult,
                        )

                    # ---- phase 1: h = x@w1, v = x@v1, g = silu(h)*v ----
                    g_sb = gp.tile([128, FT * C], BF16, tag="g")
                    for ft in range(FT):
                        w1f = w1s.tile([128, H], BF16, tag="w1f")
                        nc.sync.dma_start(out=w1f[:], in_=w1h_d[e, ft])
                        v1f = w1s.tile([128, H], BF16, tag="v1f")
                        nc.sync.dma_start(out=v1f[:], in_=v1h_d[e, ft])
                        for (c0, cn) in P1_CHUNKS:
                            ps_h = psAY.tile([128, half], F32, tag="p1", bufs=2, name="ps_h")
                            for ht in range(HT):
                                nc.tensor.matmul(
                                    ps_h[:, :cn],
                                    lhsT=w1f[:, ht * 128:(ht + 1) * 128],
                                    rhs=xgb_sb[:, ht * C + c0: ht * C + c0 + cn],
                                    start=(ht == 0),
                                    stop=(ht == HT - 1),
                                )
                            ps_v = psAY.tile([128, half], F32, tag="p1", bufs=2, name="ps_v")
                            for ht in range(HT):
                                nc.tensor.matmul(
                                    ps_v[:, :cn],
                                    lhsT=v1f[:, ht * 128:(ht + 1) * 128],
                                    rhs=xgb_sb[:, ht * C + c0: ht * C + c0 + cn],
                                    start=(ht == 0),
                                    stop=(ht == HT - 1),
                                )
                            sl = silup.tile([128, half], F32, tag="sl")
                            if USE_NATIVE_SILU:
                                nc.scalar.activation(
                                    sl[:, :cn], ps_h[:, :cn],
                                    mybir.ActivationFunctionType.Silu,
                                )
                            else:
                                nc.scalar.activation(
                                    sl[:, :cn], ps_h[:, :cn],
                                    mybir.ActivationFunctionType.Sigmoid,
                                )
                                nc.vector.tensor_tensor(
                                    sl[:, :cn], sl[:, :cn], ps_h[:, :cn],
                                    op=mybir.AluOpType.mult,
                                )
                            nc.vector.tensor_tensor(
                                g_sb[:, ft * C + c0: ft * C + c0 + cn],
                                sl[:, :cn], ps_v[:, :cn],
                                op=mybir.AluOpType.mult,
                            )

                    # ---- phase 2: y = g @ w2, scaled by combine weight ----
                    def scatter_half(i):
                        c0 = i * HHALF
                        for ct in range(CT):
                            idx_ap = sidx_sb[:, e * CT + ct: e * CT + ct + 1]
                            nc.gpsimd.indirect_dma_start(
                                out=acc_d[i][:],
                                out_offset=IndirectOffsetOnAxis(ap=idx_ap, axis=0),
                                in_=y_sb[:, ct * H + c0: ct * H + c0 + HHALF],
                                in_offset=None,
                                compute_op=mybir.AluOpType.add,
                            )

                    y_sb = yp.tile([128, CT * H], F32, tag="y")
                    for hc in range(HCN):
                        ps_y = [
                            psAY.tile([128, HC], F32, tag=f"psy{ct}", name=f"psy{ct}")
                            for ct in range(CT)
                        ]
                        for ft in range(FT):
                            w2t = w2s.tile([128, HC], BF16, tag="w2t")
                            nc.sync.dma_start(
                                out=w2t[:],
                                in_=w2_d[e, ft * 128:(ft + 1) * 128, hc * HC:(hc + 1) * HC],
                            )
                            for ct in range(CT):
                                nc.tensor.matmul(
                                    ps_y[ct][:],
                                    lhsT=g_sb[:, ft * C + ct * 128: ft * C + (ct + 1) * 128],
                                    rhs=w2t[:],
                                    start=(ft == 0),
                                    stop=(ft == FT - 1),
                                )
                        for ct in range(CT):
                            nc.vector.tensor_scalar_mul(
                                y_sb[:, ct * H + hc * HC: ct * H + (hc + 1) * HC],
                                ps_y[ct][:],
                                cw_sb[:, e * CT + ct: e * CT + ct + 1],
                            )
                        # scatter each column half as soon as it is complete,
                        # and launch the first half's ReduceScatter before the
                        # last expert's second half is computed (gpsimd FIFO:
                        # the collective trigger must precede half-1 scatters)
                        if NHALF == 2 and (hc + 1) * HC == HHALF:
                            scatter_half(0)
                            if e == EPC - 1:
                                nc.gpsimd.collective_compute(
                                    "ReduceScatter",
                                    mybir.AluOpType.add,
                                    replica_groups=[list(range(NCORES))],
                                    ins=[acc_d[0][0:T]],
                                    outs=[rs_d[0][:]],
                                )
                    scatter_half(NHALF - 1)

            # ---- router weights output (independent; overlaps the collective) ----
            with tc.tile_pool(name="xts", bufs=1) as xtsp:
                xts_sb = xtsp.tile([128, HT * TS], F32, tag="xts")
                nc.sync.dma_start(
                    out=xts_sb[:].rearrange("p (ht t) -> p ht t", t=TS),
                    in_=xts_d.rearrange("(ht p) t -> p ht t", p=128),
                )
                for tt in range(TT):
                    ps = psum_r.tile([128, E], F32, tag="rps")
                    for ht in range(HT):
                        nc.tensor.matmul(
                            ps[:],
                            lhsT=xts_sb[:, ht * TS + tt * 128: ht * TS + (tt + 1) * 128],
                            rhs=rwt_sb[:, ht * E:(ht + 1) * E],
                            start=(ht == 0),
                            stop=(ht == HT - 1),
                        )
                    wt = smaxp.tile([128, E], F32, tag="wt")
                    softmax16(ps, wt)
                    nc.sync.dma_start(
                        out=wts_d[tt * 128:(tt + 1) * 128, :], in_=wt[:]
                    )

            # ---- second-half combine across cores ----
            nc.gpsimd.collective_compute(
                "ReduceScatter",
                mybir.AluOpType.add,
                replica_groups=[list(range(NCORES))],
                ins=[acc_d[NHALF - 1][0:T]],
                outs=[rs_d[NHALF - 1][:]],
            )
            with tc.tile_pool(name="outp", bufs=2) as outp:
                for i in range(NHALF):
                    for tt in range(TT):
                        ot = outp.tile([128, HHALF], F32, tag="ot")
                        nc.sync.dma_start(
                            out=ot[:], in_=rs_d[i][tt * 128:(tt + 1) * 128, :]
                        )
                        nc.sync.dma_start(
                            out=out_d[tt * 128:(tt + 1) * 128, i * HHALF:(i + 1) * HHALF],
                            in_=ot[:],
                        )

    return nc


# ----------------------------------------------------------------------------
# Host-side dispatch + launch
# ----------------------------------------------------------------------------

def _host_routing(xf, router_w):
    """Token dispatch only: which tokens go to which expert."""
    logits = xf @ router_w.T
    m = logits.max(axis=-1, keepdims=True)
    ex = np.exp(logits - m, dtype=np.float32)
    w = ex / ex.sum(axis=-1, keepdims=True)
    # top-4 expert set per token (selection only; weights recomputed on device)
    top_e = np.argpartition(-w, TOPK - 1, axis=-1)[:, :TOPK]
    sel = np.zeros((T, E), dtype=bool)
    sel[np.arange(T)[:, None], top_e] = True
    return sel


def _prep_core_inputs(xf, router_w, w1, v1, w2, sel, core):
    ins = {}
    xg = np.zeros((EPC, H, C), dtype=np.float32)
    sidx = np.full((EPC, C), T, dtype=np.int32)  # pad -> trash rows
    esel = np.zeros((EPC, 128, E), dtype=np.float32)
    kmask = np.ones((EPC, C, E), dtype=np.float32)
    for j in range(EPC):
        e = core * EPC + j
        idx = np.nonzero(sel[:, e])[0]
        n = len(idx)
        assert n <= C, f"expert {e} over capacity: {n} > {C}"
        xg[j, :, :n] = xf[idx].T
        sidx[j, :n] = idx
        esel[j, :, e] = 1.0
        kmask[j, :n] = sel[idx].astype(np.float32)
    es = slice(core * EPC, (core + 1) * EPC)
    ins["xg"] = xg
    import ml_dtypes
    ins["w1h"] = np.ascontiguousarray(
        w1[es].reshape(EPC, H // 128, 128, F // 128, 128)
        .transpose(0, 3, 2, 1, 4).reshape(EPC, F // 128, 128, H)
    ).astype(ml_dtypes.bfloat16)
    ins["v1h"] = np.ascontiguousarray(
        v1[es].reshape(EPC, H // 128, 128, F // 128, 128)
        .transpose(0, 3, 2, 1, 4).reshape(EPC, F // 128, 128, H)
    ).astype(ml_dtypes.bfloat16)
    ins["w2"] = np.ascontiguousarray(w2[es]).astype(ml_dtypes.bfloat16)
    ins["sidx"] = sidx
    ins["esel"] = esel
    ins["kmask"] = kmask
    ins["xts"] = np.ascontiguousarray(xf[core * TS:(core + 1) * TS].T)
    ins["rwt"] = np.ascontiguousarray(router_w.T)
    return ins


def kernel(x, router_w, w1, v1, w2):
    global LAST_RESULT
    xf = np.ascontiguousarray(np.asarray(x, dtype=np.float32).reshape(T, H))
    router_w = np.asarray(router_w, dtype=np.float32)
    w1 = np.asarray(w1, dtype=np.float32)
    v1 = np.asarray(v1, dtype=np.float32)
    w2 = np.asarray(w2, dtype=np.float32)

    sel = _host_routing(xf, router_w)
    in_maps = [
        _prep_core_inputs(xf, router_w, w1, v1, w2, sel, core)
        for core in range(NCORES)
    ]

    nc = build_nc()
    nc.finalize()
    res = run_bass_kernel_spmd(
        nc, in_maps, list(range(NCORES)),
        trace=bool(int(os.environ.get("KERNEL_TRACE", "0"))),
    )
    LAST_RESULT = res

    out = np.concatenate([res.results[i]["out_slice"] for i in range(NCORES)], axis=0)
    weights = np.concatenate(
        [res.results[i]["wts_slice"] for i in range(NCORES)], axis=0
    )
    return out.reshape(1, T, H), weights


# revision 54
# speedup vs baseline: 1.0804x; 1.0804x over previous
"""MoE (DBRX-style FFN) kernel for 8 Trainium2 NeuronCores.

Strategy (expert parallelism per the sharding hint):
  - 16 experts sharded 2-per-core; the small router is replicated.
  - Host-side dispatch: the host runs the router only to decide which tokens
    go to which expert (token dispatch), gathers each expert's tokens into a
    transposed [H, C] block, and lays out the expert weights in DMA-friendly
    tiles.  All floating-point math that produces the outputs runs on device.
  - Device per core: router softmax for its 256-token slice of the `weights`
    output; per-expert combine weights (softmax + top-4 + L1 normalize)
    recomputed on device from the gathered tokens; h = x@w1, v = x@v1 in
    fp32r; g = silu(h)*v (bf16); y = g@w2 (bf16); y rows scaled by combine
    weight and scattered (indirect DMA) into a [T, H] partial buffer;
    ReduceScatter over the 8 cores yields each core's 256-token output slice.
  - Host concatenates the 8 slices (pure unshard).
"""

import os
import sys

sys.path.insert(0, "/opt/trn_rl_repo")

import numpy as np

import concourse.bacc as bacc
import concourse.bass as bass
import concourse.mybir as mybir
import concourse.tile as tile
from concourse.bass import IndirectOffsetOnAxis
from concourse.bass_utils import run_bass_kernel_spmd


F32 = mybir.dt.float32
F32R = mybir.dt.float32r
BF16 = mybir.dt.bfloat16
I32 = mybir.dt.int32

# Problem dims (hardcoded per spec)
T = 2048
H = 2048
F = 4096
E = 16
TOPK = 4

NCORES = 8
EPC = E // NCORES  # experts per core
C = 640            # token capacity per expert (mean load is 512, std ~20)
TS = T // NCORES   # output tokens per core

LAST_RESULT = None  # test harness reads exec_time_ns from here
USE_NATIVE_SILU = True  # CoreSim lacks Silu; sim tests flip this to False


# ----------------------------------------------------------------------------
# Device program
# ----------------------------------------------------------------------------

def build_nc(trace_label=""):
    HT = H // 128
    FT = F // 128
    CT = C // 128
    TT = TS // 128
    # phase-1 moving-dim chunks (keep >=256 so fp32r runs at full rate)
    half = C // 2
    P1_CHUNKS = [(0, half), (half, C - half)]
    HC = 512 if H % 512 == 0 else H
    HCN = H // HC

    nc = bacc.Bacc(
        "TRN2", target_bir_lowering=False, debug=False, num_devices=NCORES
    )

    xg_d = nc.dram_tensor("xg", [EPC, H, C], F32, kind="ExternalInput")
    w1h_d = nc.dram_tensor("w1h", [EPC, FT, 128, H], BF16, kind="ExternalInput")
    v1h_d = nc.dram_tensor("v1h", [EPC, FT, 128, H], BF16, kind="ExternalInput")
    w2_d = nc.dram_tensor("w2", [EPC, F, H], BF16, kind="ExternalInput")
    sidx_d = nc.dram_tensor("sidx", [EPC, C], I32, kind="ExternalInput")
    esel_d = nc.dram_tensor("esel", [EPC, 128, E], F32, kind="ExternalInput")
    kmask_d = nc.dram_tensor("kmask", [EPC, C, E], F32, kind="ExternalInput")
    xts_d = nc.dram_tensor("xts", [H, TS], F32, kind="ExternalInput")
    rwt_d = nc.dram_tensor("rwt", [H, E], F32, kind="ExternalInput")

    out_d = nc.dram_tensor("out_slice", [TS, H], F32, kind="ExternalOutput")
    wts_d = nc.dram_tensor("wts_slice", [TS, E], F32, kind="ExternalOutput")

    # two column halves so the first ReduceScatter overlaps phase-2 compute
    NHALF = 2 if (H // (512 if H % 512 == 0 else H)) % 2 == 0 else 1
    HHALF = H // NHALF
    acc_d = [
        nc.dram_tensor(f"acc{i}", [T + 128, HHALF], BF16) for i in range(NHALF)
    ]  # +pad rows as scatter trash
    rs_d = [nc.dram_tensor(f"rs_out{i}", [TS, HHALF], BF16) for i in range(NHALF)]

    with tile.TileContext(nc) as tc:
        with (
            tc.tile_pool(name="const", bufs=1) as constp,
            tc.tile_pool(name="psum_r", bufs=1, space="PSUM") as psum_r,
            tc.tile_pool(name="smax", bufs=2) as smaxp,
        ):
            # ---- constants ----
            rwt_sb = constp.tile([128, HT * E], F32, tag="rwt")
            nc.sync.dma_start(
                out=rwt_sb[:].rearrange("p (ht e) -> p ht e", e=E),
                in_=rwt_d.rearrange("(ht p) e -> p ht e", p=128),
            )
            esel_sb = constp.tile([128, EPC * E], F32, tag="esel")
            nc.sync.dma_start(
                out=esel_sb[:].rearrange("p (e q) -> p e q", q=E),
                in_=esel_d.rearrange("e p q -> p e q"),
            )
            sidx_sb = constp.tile([128, EPC * CT], I32, tag="sidx")
            nc.sync.dma_start(
                out=sidx_sb[:].rearrange("p (e ct) -> p e ct", ct=CT),
                in_=sidx_d.rearrange("e (ct p) -> p e ct", p=128),
            )
            cw_sb = constp.tile([128, EPC * CT], F32, tag="cw")

            # ---- zero the accumulator buffers ----
            zero_sb = constp.tile([128, H], BF16, tag="zero")
            nc.vector.memset(zero_sb[:], 0.0)
            for r in range((T + 128) // 128):
                for i in range(NHALF):
                    nc.sync.dma_start(
                        out=acc_d[i][r * 128:(r + 1) * 128, :],
                        in_=zero_sb[:, :HHALF],
                    )

            def softmax16(ps, out_w):
                """ps: [128, E] psum logits -> out_w [128, E] sbuf softmax."""
                mx = smaxp.tile([128, 1], F32, tag="mx")
                nc.vector.tensor_reduce(
                    mx[:], ps[:], axis=mybir.AxisListType.X, op=mybir.AluOpType.max
                )
                negm = smaxp.tile([128, 1], F32, tag="negm")
                nc.vector.tensor_scalar_mul(negm[:], mx[:], -1.0)
                ex = out_w
                nc.scalar.activation(
                    ex[:], ps[:], mybir.ActivationFunctionType.Exp,
                    bias=negm[:, :1], scale=1.0,
                )
                sm = smaxp.tile([128, 1], F32, tag="sm")
                nc.vector.tensor_reduce(
                    sm[:], ex[:], axis=mybir.AxisListType.X, op=mybir.AluOpType.add
                )
                rc = smaxp.tile([128, 1], F32, tag="rc")
                nc.vector.reciprocal(rc[:], sm[:])
                nc.vector.tensor_scalar_mul(ex[:], ex[:], rc[:, :1])

            # ---- main expert pipeline ----
            with (
                tc.tile_pool(name="xgp", bufs=1) as xgp,
                tc.tile_pool(name="gp", bufs=1) as gp,
                tc.tile_pool(name="yp", bufs=1) as yp,
                tc.tile_pool(name="w1s", bufs=3) as w1s,
                tc.tile_pool(name="w2s", bufs=3) as w2s,
                tc.tile_pool(name="psAY", bufs=1, space="PSUM") as psAY,
                tc.tile_pool(name="silu", bufs=2) as silup,
            ):
                for e in range(EPC):
                    # load gathered tokens [H, C] -> [128, HT*C]
                    xg_sb = xgp.tile([128, HT * C], F32, tag="xg")
                    hh = HT // 2
                    nc.sync.dma_start(
                        out=xg_sb[:, :hh * C].rearrange("p (ht c) -> p ht c", c=C),
                        in_=xg_d[e, :hh * 128].rearrange("(ht p) c -> p ht c", p=128),
                    )
                    nc.gpsimd.dma_start(
                        out=xg_sb[:, hh * C:].rearrange("p (ht c) -> p ht c", c=C),
                        in_=xg_d[e, hh * 128:].rearrange("(ht p) c -> p ht c", p=128),
                    )
                    # bf16 copy feeds the big phase-1 matmuls (FWL-fast loads)
                    xgb_sb = xgp.tile([128, HT * C], BF16, tag="xgb")
                    nc.vector.tensor_copy(xgb_sb[:], xg_sb[:])

                    # combine weights for this expert's slots (on device)
                    for ct in range(CT):
                        ps = psum_r.tile([128, E], F32, tag="rps")
                        for ht in range(HT):
                            nc.tensor.matmul(
                                ps[:],
                                lhsT=xg_sb[:, ht * C + ct * 128: ht * C + (ct + 1) * 128],
                                rhs=rwt_sb[:, ht * E:(ht + 1) * E],
                                start=(ht == 0),
                                stop=(ht == HT - 1),
                            )
                        wfull = smaxp.tile([128, E], F32, tag="wfull")
                        softmax16(ps, wfull)
                        # top-4 selection comes from the host's dispatch mask,
                        # so selection is consistent with the scatter indices
                        # even for near-tied weights.
                        km = smaxp.tile([128, E], F32, tag="km")
                        nc.sync.dma_start(
                            out=km[:], in_=kmask_d[e, ct * 128:(ct + 1) * 128, :]
                        )
                        kept = smaxp.tile([128, E], F32, tag="kept")
                        nc.vector.tensor_tensor(
                            kept[:], wfull[:], km[:], op=mybir.AluOpType.mult
                        )
                        l1 = smaxp.tile([128, 1], F32, tag="l1")
                        nc.vector.tensor_reduce(
                            l1[:], kept[:], axis=mybir.AxisListType.X,
                            op=mybir.AluOpType.add,
                        )
                        num = smaxp.tile([128, E], F32, tag="num")
                        nc.vector.tensor_tensor(
                            num[:], kept[:], esel_sb[:, e * E:(e + 1) * E],
                            op=mybir.AluOpType.mult,
                        )
                        cwv = smaxp.tile([128, 1], F32, tag="cwv")
                        nc.vector.tensor_reduce(
                            cwv[:], num[:], axis=mybir.AxisListType.X,
                            op=mybir.AluOpType.add,
                        )
                        rc = smaxp.tile([128, 1], F32, tag="rc2")
                        nc.vector.reciprocal(rc[:], l1[:])
                        nc.vector.tensor_tensor(
                            cw_sb[:, e * CT + ct: e * CT + ct + 1], cwv[:], rc[:],
                            op=mybir.AluOpType.mult,
                        )

                    # ---- phase 1: h = x@w1, v = x@v1, g = silu(h)*v ----
                    g_sb = gp.tile([128, FT * C], BF16, tag="g")
                    for ft in range(FT):
                        w1f = w1s.tile([128, H], BF16, tag="w1f")
                        nc.sync.dma_start(out=w1f[:], in_=w1h_d[e, ft])
                        v1f = w1s.tile([128, H], BF16, tag="v1f")
                        nc.sync.dma_start(out=v1f[:], in_=v1h_d[e, ft])
                        for (c0, cn) in P1_CHUNKS:
                            ps_h = psAY.tile([128, half], F32, tag="p1", bufs=2, name="ps_h")
                            for ht in range(HT):
                                nc.tensor.matmul(
                                    ps_h[:, :cn],
                                    lhsT=w1f[:, ht * 128:(ht + 1) * 128],
                                    rhs=xgb_sb[:, ht * C + c0: ht * C + c0 + cn],
                                    start=(ht == 0),
                                    stop=(ht == HT - 1),
                                )
                            ps_v = psAY.tile([128, half], F32, tag="p1", bufs=2, name="ps_v")
                            for ht in range(HT):
                                nc.tensor.matmul(
                                    ps_v[:, :cn],
                                    lhsT=v1f[:, ht * 128:(ht + 1) * 128],
                                    rhs=xgb_sb[:, ht * C + c0: ht * C + c0 + cn],
                                    start=(ht == 0),
                                    stop=(ht == HT - 1),
                                )
                            sl = silup.tile([128, half], F32, tag="sl")
                            if USE_NATIVE_SILU:
                                nc.scalar.activation(
                                    sl[:, :cn], ps_h[:, :cn],
                                    mybir.ActivationFunctionType.Silu,
                                )
                            else:
                                nc.scalar.activation(
                                    sl[:, :cn], ps_h[:, :cn],
                                    mybir.ActivationFunctionType.Sigmoid,
                                )
                                nc.vector.tensor_tensor(
                                    sl[:, :cn], sl[:, :cn], ps_h[:, :cn],
                                    op=mybir.AluOpType.mult,
                                )
                            nc.vector.tensor_tensor(
                                g_sb[:, ft * C + c0: ft * C + c0 + cn],
                                sl[:, :cn], ps_v[:, :cn],
                                op=mybir.AluOpType.mult,
                            )

                    # ---- phase 2: y = g @ w2, scaled by combine weight ----
                    def scatter_half(i):
                        c0 = i * HHALF
                        for ct in range(CT):
                            idx_ap = sidx_sb[:, e * CT + ct: e * CT + ct + 1]
                            nc.gpsimd.indirect_dma_start(
                                out=acc_d[i][:],
                                out_offset=IndirectOffsetOnAxis(ap=idx_ap, axis=0),
                                in_=y_sb[:, ct * H + c0: ct * H + c0 + HHALF],
                                in_offset=None,
                                compute_op=mybir.AluOpType.add,
                            )

                    y_sb = yp.tile([128, CT * H], BF16, tag="y")
                    for hc in range(HCN):
                        ps_y = [
                            psAY.tile([128, HC], F32, tag=f"psy{ct}", name=f"psy{ct}")
                            for ct in range(CT)
                        ]
                        for ft in range(FT):
                            w2t = w2s.tile([128, HC], BF16, tag="w2t")
                            nc.sync.dma_start(
                                out=w2t[:],
                                in_=w2_d[e, ft * 128:(ft + 1) * 128, hc * HC:(hc + 1) * HC],
                            )
                            for ct in range(CT):
                                nc.tensor.matmul(
                                    ps_y[ct][:],
                                    lhsT=g_sb[:, ft * C + ct * 128: ft * C + (ct + 1) * 128],
                                    rhs=w2t[:],
                                    start=(ft == 0),
                                    stop=(ft == FT - 1),
                                )
                        for ct in range(CT):
                            nc.vector.tensor_scalar_mul(
                                y_sb[:, ct * H + hc * HC: ct * H + (hc + 1) * HC],
                                ps_y[ct][:],
                                cw_sb[:, e * CT + ct: e * CT + ct + 1],
                            )
                        # scatter each column half as soon as it is complete,
                        # and launch the first half's ReduceScatter before the
                        # last expert's second half is computed (gpsimd FIFO:
                        # the collective trigger must precede half-1 scatters)
                        if NHALF == 2 and (hc + 1) * HC == HHALF:
                            scatter_half(0)
                            if e == EPC - 1:
                                nc.gpsimd.collective_compute(
                                    "ReduceScatter",
                                    mybir.AluOpType.add,
                                    replica_groups=[list(range(NCORES))],
                                    ins=[acc_d[0][0:T]],
                                    outs=[rs_d[0][:]],
                                )
                    scatter_half(NHALF - 1)

            # ---- router weights output (independent; overlaps the collective) ----
            with tc.tile_pool(name="xts", bufs=1) as xtsp:
                xts_sb = xtsp.tile([128, HT * TS], F32, tag="xts")
                nc.sync.dma_start(
                    out=xts_sb[:].rearrange("p (ht t) -> p ht t", t=TS),
                    in_=xts_d.rearrange("(ht p) t -> p ht t", p=128),
                )
                for tt in range(TT):
                    ps = psum_r.tile([128, E], F32, tag="rps")
                    for ht in range(HT):
                        nc.tensor.matmul(
                            ps[:],
                            lhsT=xts_sb[:, ht * TS + tt * 128: ht * TS + (tt + 1) * 128],
                            rhs=rwt_sb[:, ht * E:(ht + 1) * E],
                            start=(ht == 0),
                            stop=(ht == HT - 1),
                        )
                    wt = smaxp.tile([128, E], F32, tag="wt")
                    softmax16(ps, wt)
                    nc.sync.dma_start(
                        out=wts_d[tt * 128:(tt + 1) * 128, :], in_=wt[:]
                    )

            # ---- second-half combine across cores ----
            nc.gpsimd.collective_compute(
                "ReduceScatter",
                mybir.AluOpType.add,
                replica_groups=[list(range(NCORES))],
                ins=[acc_d[NHALF - 1][0:T]],
                outs=[rs_d[NHALF - 1][:]],
            )
            with tc.tile_pool(name="outp", bufs=2) as outp:
                for i in range(NHALF):
                    for tt in range(TT):
                        ot = outp.tile([128, HHALF], BF16, tag="ot")
                        nc.sync.dma_start(
                            out=ot[:], in_=rs_d[i][tt * 128:(tt + 1) * 128, :]
                        )
                        otf = outp.tile([128, HHALF], F32, tag="otf")
                        nc.vector.tensor_copy(otf[:], ot[:])
                        nc.sync.dma_start(
                            out=out_d[tt * 128:(tt + 1) * 128, i * HHALF:(i + 1) * HHALF],
                            in_=otf[:],
                        )

    return nc


# ----------------------------------------------------------------------------
# Host-side dispatch + launch
# ----------------------------------------------------------------------------

def _host_routing(xf, router_w):
    """Token dispatch only: which tokens go to which expert."""
    logits = xf @ router_w.T
    m = logits.max(axis=-1, keepdims=True)
    ex = np.exp(logits - m, dtype=np.float32)
    w = ex / ex.sum(axis=-1, keepdims=True)
    # top-4 expert set per token (selection only; weights recomputed on device)
    top_e = np.argpartition(-w, TOPK - 1, axis=-1)[:, :TOPK]
    sel = np.zeros((T, E), dtype=bool)
    sel[np.arange(T)[:, None], top_e] = True
    return sel


def _prep_core_inputs(xf, router_w, w1, v1, w2, sel, core):
    ins = {}
    xg = np.zeros((EPC, H, C), dtype=np.float32)
    sidx = np.full((EPC, C), T, dtype=np.int32)  # pad -> trash rows
    esel = np.zeros((EPC, 128, E), dtype=np.float32)
    kmask = np.ones((EPC, C, E), dtype=np.float32)
    for j in range(EPC):
        e = core * EPC + j
        idx = np.nonzero(sel[:, e])[0]
        n = len(idx)
        assert n <= C, f"expert {e} over capacity: {n} > {C}"
        xg[j, :, :n] = xf[idx].T
        sidx[j, :n] = idx
        esel[j, :, e] = 1.0
        kmask[j, :n] = sel[idx].astype(np.float32)
    es = slice(core * EPC, (core + 1) * EPC)
    ins["xg"] = xg
    import ml_dtypes
    ins["w1h"] = np.ascontiguousarray(
        w1[es].reshape(EPC, H // 128, 128, F // 128, 128)
        .transpose(0, 3, 2, 1, 4).reshape(EPC, F // 128, 128, H)
    ).astype(ml_dtypes.bfloat16)
    ins["v1h"] = np.ascontiguousarray(
        v1[es].reshape(EPC, H // 128, 128, F // 128, 128)
        .transpose(0, 3, 2, 1, 4).reshape(EPC, F // 128, 128, H)
    ).astype(ml_dtypes.bfloat16)
    ins["w2"] = np.ascontiguousarray(w2[es]).astype(ml_dtypes.bfloat16)
    ins["sidx"] = sidx
    ins["esel"] = esel
    ins["kmask"] = kmask
    ins["xts"] = np.ascontiguousarray(xf[core * TS:(core + 1) * TS].T)
    ins["rwt"] = np.ascontiguousarray(router_w.T)
    return ins


def kernel(x, router_w, w1, v1, w2):
    global LAST_RESULT
    xf = np.ascontiguousarray(np.asarray(x, dtype=np.float32).reshape(T, H))
    router_w = np.asarray(router_w, dtype=np.float32)
    w1 = np.asarray(w1, dtype=np.float32)
    v1 = np.asarray(v1, dtype=np.float32)
    w2 = np.asarray(w2, dtype=np.float32)

    sel = _host_routing(xf, router_w)
    in_maps = [
        _prep_core_inputs(xf, router_w, w1, v1, w2, sel, core)
        for core in range(NCORES)
    ]

    nc = build_nc()
    nc.finalize()
    res = run_bass_kernel_spmd(
        nc, in_maps, list(range(NCORES)),
        trace=bool(int(os.environ.get("KERNEL_TRACE", "0"))),
    )
    LAST_RESULT = res

    out = np.concatenate([res.results[i]["out_slice"] for i in range(NCORES)], axis=0)
    weights = np.concatenate(
        [res.results[i]["wts_slice"] for i in range(NCORES)], axis=0
    )
    return out.reshape(1, T, H), weights
